# revision 33
# baseline (speedup 1.0000x reference)
"""Trainium2 Bass kernel for nn_Decoder_68289980006849 (3-layer transformer decoder).

Strategy: data-parallel over batch (B=8) across 8 NeuronCores; zero collectives.
Per core, the full decoder runs in "T-layout" [feature(partitions), token(free)]
with an fp16 datapath (fp32 PSUM accumulation):

  - x = embed-gather*sqrt(D) + pe precomputed on host (input sharding)
  - LayerNorm split three ways: per-chunk stats matmuls are hoisted into the
    previous phase's residual evictions; projections run on (x - mean) as soon
    as the mean lands; the 1/std factor is computed in parallel and folded
    into each projection's PSUM eviction (for the FFN it rides the FFN2
    eviction, exact since relu(r*z) = r*relu(z) for r > 0)
  - attention: scores computed transposed (scores_T[tk, tq]) so the softmaxed
    matrix feeds the context matmul directly; heads are software-pipelined
    (head h-1's context matmuls interleave MM-by-MM with head h's score
    matmuls); score chunk-pairs land in one 2-bank fp32 PSUM tile and get a
    single wide ACT exp; 64 ones-columns prepended to V give the softmax
    denominator rows at PSUM partition 0, where the fast reciprocal reads
    PSUM directly; one fused TT normalizes each head's context; causal
    masking via column-slicing + diagonal-block multiply on GpSimd
  - cross-attn K/V projections run as fillers inside the self-attn SDP phase
  - FFN: dense FFN1 stream, then FFN2 j-major with full PSUM accumulation
    (one fused eviction per d-chunk incl. the reference quirk x = cross + ffn);
    cross-attn output stays resident in SBUF
  - final transposes interleave with the last layer's FFN2 evictions
  - weights pre-packed on host as fp16 tile images (contiguous 0.5-1MB DMAs);
    fp16 halves HBM traffic and enables fast weight load on the PE

Self-contained: only stdlib + numpy + the concourse/bass stack on PYTHONPATH.
"""

import os
import numpy as np

import concourse.bass as bass
import concourse.tile as tile
from concourse import bacc, mybir
from concourse.masks import make_identity

# ---- problem constants (hardcoded per contract) ----
B, LD, LE = 8, 512, 512
D, H, DK, F, L, V = 1024, 16, 64, 4096, 3, 32000
M = LD                      # tokens per core
DCH = D // 128              # 8 d-model chunks
FCH = F // 128              # 32 ffn chunks
MCH = M // 128              # 4 token chunks
SQRT_D = 32.0
INV_SQRT_DK = 0.125
EPS = 1e-5
NONES = 64                  # ones-columns appended to V (denominator rows)

P = 128
N = 512
F32 = mybir.dt.float32
F16 = mybir.dt.float16
AF = mybir.ActivationFunctionType
ALU = mybir.AluOpType

_CACHE = {}


# ----------------------------------------------------------------------------
# Bass program (identical on all 8 cores; data differs via in_maps)
# ----------------------------------------------------------------------------

def _build_nc():
    nc = bacc.Bacc("TRN2", target_bir_lowering=False, debug=False,
                   enable_asserts=False, num_devices=8)

    # inputs (per core); x0p = embed-gather * sqrt(D) + pe, done on host
    x0p = nc.dram_tensor("x0p", [P, DCH, N], F16, kind="ExternalInput").ap()
    encp = nc.dram_tensor("encp", [P, DCH, N], F16, kind="ExternalInput").ap()
    # projection weights, packed: [l, a, i(q,k,v,o), g, 128, 8, 512]
    wp = nc.dram_tensor("wp", [L, 2, 4, 2, P, DCH, N], F16, kind="ExternalInput").ap()
    w1p = nc.dram_tensor("w1p", [L, 8, P, DCH, N], F16, kind="ExternalInput").ap()
    # FFN2 weights j-major: [l, j, 128(f), 8(kb), 4(ki), 128(d)]
    w2p = nc.dram_tensor("w2p", [L, 8, P, 8, 4, P], F16, kind="ExternalInput").ap()
    causal = nc.dram_tensor("causal", [P, P], F16, kind="ExternalInput").ap()
    out = nc.dram_tensor("out", [M, D], F32, kind="ExternalOutput").ap()

    with tile.TileContext(nc) as tc:
        with tc.tile_pool(name="res", bufs=1) as res, \
             tc.tile_pool(name="wpool", bufs=3) as wpool, \
             tc.tile_pool(name="spool", bufs=2) as spool, \
             tc.tile_pool(name="psum", bufs=1, space="PSUM") as psum:

            # ---- resident tiles ----
            xT = res.tile([P, DCH, N], F16)         # residual stream
            encT = res.tile([P, DCH, N], F16)       # encoder output (transposed)
            Vst = res.tile([P, MCH, H, 64 + NONES], F16)  # V + ones columns
            Jsc = res.tile([P, P], F16)             # all-(1/D) for LN stats
            c01 = res.tile([P, P], F16)             # causal diagonal 0/1 keep-mask
            ident = res.tile([P, P], F16)
            epsc = res.tile([P, 1], F32)

            nc.vector.memset(epsc[:], EPS)
            nc.vector.memset(Jsc[:], 1.0 / D)
            # ones-columns FIRST: the denominator rows then land at PSUM
            # partition base 0, where the direct-from-PSUM reciprocal works
            nc.vector.memset(Vst[:, :, :, 0:NONES], 1.0)

            # ---- x = embed*sqrt(D) + pe, precomputed on host; per-chunk DMA
            # so the first LayerNorm stats can start early ----
            for c in range(DCH):
                nc.sync.dma_start(xT[:, c], x0p[:, c])

            # needed from the first SDP phase onwards, not at start
            nc.sync.dma_start(encT[:], encp)
            nc.sync.dma_start(c01[:], causal)
            make_identity(nc, ident[:])

            work = tc.alloc_tile_pool(name="work", bufs=1)
            hT = work.tile([P, DCH, N], F16, tag="hT")
            hS = work.tile([P, DCH, N], F16, tag="hS")
            ctxT = work.tile([P, DCH, N], F16, tag="ctxT")
            qT = work.tile([P, DCH, N], F16, tag="qT")
            kT = work.tile([P, DCH, N], F16, tag="kT")
            cT = work.tile([P, DCH, N], F16, tag="cT")    # cross-attn output
            uT = work.tile([P, FCH, N], F16, tag="uT")    # relu(ffn1) activations

            # ---- LayerNorm split: stats (hoistable) + finish (chain+apply) ----
            def ln_begin():
                mean_ps = psum.tile([P, N], F32, tag="ctx", bufs=2, name="mean_ps")
                msq_ps = psum.tile([P, N], F32, tag="ctx", bufs=2, name="msq_ps")

                def stat_chunk(c):
                    sq = spool.tile([P, N], F16, tag="sq", bufs=2, name="sq")
                    nc.scalar.activation(sq[:], xT[:, c], AF.Square)
                    nc.tensor.matmul(mean_ps[:], Jsc[:], xT[:, c],
                                     start=(c == 0), stop=(c == DCH - 1),
                                     skip_group_check=True)
                    nc.tensor.matmul(msq_ps[:], Jsc[:], sq[:],
                                     start=(c == 0), stop=(c == DCH - 1),
                                     skip_group_check=True)
                return mean_ps, msq_ps, stat_chunk

            def ln_mid(dst, mean_ps):
                """dst = x - mean: the minimal chain before projections can
                start (r-scaling is folded into their evictions)."""
                mSB = spool.tile([P, N], F16, tag="stt", name="mSB")
                nc.scalar.activation(mSB[:], mean_ps[:], AF.Copy)
                for c in range(DCH):
                    nc.vector.tensor_tensor(dst[:, c], xT[:, c], mSB[:],
                                            op=ALU.subtract)

            def ln_rchain(mean_ps, msq_ps):
                """rstd (fp16, broadcast) - overlaps the next phase's matmuls."""
                m2 = spool.tile([P, N], F32, tag="stt32", name="m2")
                nc.scalar.activation(m2[:], mean_ps[:], AF.Square)
                var = spool.tile([P, N], F32, tag="stt32", name="var")
                nc.vector.tensor_tensor(var[:], msq_ps[:], m2[:], op=ALU.subtract)
                sd = spool.tile([P, N], F32, tag="stt32", name="sd")
                nc.scalar.activation(sd[:], var[:], AF.Sqrt, bias=epsc[:])
                rstd = spool.tile([P, N], F32, tag="stt32", name="rstd")
                nc.vector.reciprocal_approx_fast(rstd[:], sd[:])
                rstdh = spool.tile([P, N], F16, tag="stt", name="rstdh")
                nc.vector.tensor_copy(rstdh[:], rstd[:])
                return rstdh

            def proj(rhs, w_groups, evict, post=None):
                """out[j] = sum_k W[k, j-chunk].T @ rhs[k]; W streamed in 1MB
                tiles, landing in column halves for early start."""
                for g in range(2):
                    wt = wpool.tile([P, DCH, N], F16, tag="wt", name="wt")
                    nc.sync.dma_start(wt[:, :, 0:N // 2], w_groups[g][:, :, 0:N // 2])
                    nc.sync.dma_start(wt[:, :, N // 2:], w_groups[g][:, :, N // 2:])
                    for jj in range(4):
                        j = g * 4 + jj
                        ps = psum.tile([P, N], F32, tag="mm", bufs=2, name="ps_mm")
                        for k in range(DCH):
                            nc.tensor.matmul(ps[:], wt[:, k, jj * P:(jj + 1) * P],
                                             rhs[:, k], start=(k == 0),
                                             stop=(k == DCH - 1))
                        evict(j, ps)
                        if post is not None:
                            post(j)

            def v_proj(kv_rhs, w_groups):
                """V_nat[tk, dv] -> Vst[:, t, h, :64] slices."""
                for g in range(2):  # dv halves (heads g*8..g*8+7)
                    wt = wpool.tile([P, DCH, N], F16, tag="wt", name="wt")
                    nc.sync.dma_start(wt[:], w_groups[g])
                    for t in range(MCH):
                        ps = psum.tile([P, N], F32, tag="mm", bufs=2, name="ps_v")
                        for k in range(DCH):
                            nc.tensor.matmul(ps[:], kv_rhs[:, k, t * P:(t + 1) * P],
                                             wt[:, k], start=(k == 0),
                                             stop=(k == DCH - 1))
                        nc.scalar.activation(
                            Vst[:, t, g * 8:(g + 1) * 8, NONES:NONES + 64],
                            ps[:].rearrange("p (h d) -> p h d", d=64), AF.Copy)

            # ---- attention: scores/ctx split for head software-pipelining ----
            def sdp_ctx_mm(h, exps, is_self, ctx, c):
                nc.tensor.matmul(ctx[:, (c * P if is_self else 0):],
                                 Vst[:, c, h, :],
                                 exps[c][:, (c * P if is_self else 0):],
                                 start=(c == 0), stop=(c == MCH - 1),
                                 skip_group_check=True)

            def sdp_ctx_fin(h, ctx):
                hc, off = h // 2, 64 * (h % 2)
                rec = spool.tile([NONES, N], F32, tag="rec", bufs=2, name="rec")
                nc.vector.reciprocal_approx_fast(rec[:], ctx[0:NONES, :])
                nc.vector.tensor_tensor(ctxT[off:off + 64, hc, :],
                                        ctx[NONES:NONES + 64, :],
                                        rec[:], op=ALU.mult)

            def ctx_tile():
                return psum.tile([P, N], F32, tag="ctx", bufs=2, name="ctx")

            for l in range(L):
                def wgrp(a, i):
                    return [wp[l, a, i, g] for g in range(2)]

                # ---- self-attention ----
                if l == 0:
                    ln1_state = ln_begin()
                    for c in range(DCH):
                        ln1_state[2](c)
                ln_mid(hT, ln1_state[0])
                r1 = ln_rchain(ln1_state[0], ln1_state[1])

                def ev_scale(dst, r):
                    return lambda j, ps: nc.vector.tensor_tensor(
                        dst[:, j], ps[:], r[:], op=ALU.mult)
                proj(hT, wgrp(0, 0), ev_scale(qT, r1))
                # hS = full normalized h (for the V projection, which needs
                # per-token scaling on its matmul INPUT side)
                for c in range(DCH):
                    nc.vector.tensor_tensor(hS[:, c], hT[:, c], r1[:],
                                            op=ALU.mult)
                proj(hT, wgrp(0, 1), ev_scale(kT, r1))
                v_proj(hS, wgrp(0, 2))

                # cross-attn K/V projections depend only on encT + weights ->
                # emitted as fillers inside the self-attention SDP.
                ck_state = {}
                wkx, wvx = wgrp(1, 1), wgrp(1, 2)

                def ck_dma(g):
                    def f():
                        wt = wpool.tile([P, DCH, N], F16, tag="wt", name="wt")
                        nc.sync.dma_start(wt[:, :, 0:N // 2], wkx[g][:, :, 0:N // 2])
                        nc.sync.dma_start(wt[:, :, N // 2:], wkx[g][:, :, N // 2:])
                        ck_state["k"] = wt
                    return f

                def ck_chunk(g, jj):
                    def f():
                        wt = ck_state["k"]
                        j = g * 4 + jj
                        ps = psum.tile([P, N], F32, tag="mm", bufs=2, name="ps_ck")
                        for k in range(DCH):
                            nc.tensor.matmul(ps[:], wt[:, k, jj * P:(jj + 1) * P],
                                             encT[:, k], start=(k == 0),
                                             stop=(k == DCH - 1))
                        nc.vector.tensor_copy(kT[:, j], ps[:])
                    return f

                def cv_dma(g):
                    def f():
                        wt = wpool.tile([P, DCH, N], F16, tag="wt", name="wt")
                        nc.sync.dma_start(wt[:], wvx[g])
                        ck_state["v"] = wt
                    return f

                def cv_chunk(g, t):
                    def f():
                        wt = ck_state["v"]
                        ps = psum.tile([P, N], F32, tag="mm", bufs=2, name="ps_cv")
                        for k in range(DCH):
                            nc.tensor.matmul(ps[:], encT[:, k, t * P:(t + 1) * P],
                                             wt[:, k], start=(k == 0),
                                             stop=(k == DCH - 1))
                        nc.vector.tensor_copy(
                            Vst[:, t, g * 8:(g + 1) * 8, NONES:NONES + 64],
                            ps[:].rearrange("p (h d) -> p h d", d=64))
                    return f

                # kT[:, j] may be overwritten only after heads 2j, 2j+1 read
                # it; Vst heads g*8..g*8+7 only after those heads' ctx matmuls.
                fillers = {
                    0: [ck_dma(0)],
                    2: [ck_chunk(0, 0)], 4: [ck_chunk(0, 1)],
                    6: [ck_chunk(0, 2)], 8: [ck_chunk(0, 3), ck_dma(1)],
                    10: [ck_chunk(1, 0), cv_dma(0)],
                    12: [ck_chunk(1, 1), cv_chunk(0, 0)],
                    13: [cv_chunk(0, 1)],
                    14: [ck_chunk(1, 2), cv_chunk(0, 2)],
                    15: [cv_chunk(0, 3)],
                    16: [ck_chunk(1, 3), cv_dma(1),
                         cv_chunk(1, 0), cv_chunk(1, 1),
                         cv_chunk(1, 2), cv_chunk(1, 3)],
                }

                # head-pipelined self SDP: head h-1's ctx matmuls interleave
                # MM-by-MM with head h's score matmuls (hides the short-score
                # LDWEIGHTS exposure and the exp chain)
                pend = None
                for h in range(H):
                    for cb in fillers.get(h, ()):
                        cb()
                    hc, off = h // 2, 64 * (h % 2)
                    exps = []
                    if pend is not None:
                        pend[2] = ctx_tile()
                    for cp in range(2):  # chunk pairs in one 2-bank PSUM tile
                        sc = psum.tile([P, 2, N], F32, tag="sc", bufs=2,
                                       name="sc")
                        for c2 in range(2):
                            c = 2 * cp + c2
                            cs = c * P
                            nc.tensor.matmul(sc[:, c2, cs:],
                                             kT[off:off + 64, hc,
                                                c * P:(c + 1) * P],
                                             qT[off:off + 64, hc, cs:],
                                             start=True, stop=True)
                            if pend is not None:
                                sdp_ctx_mm(pend[0], pend[1], True, pend[2], c)
                        ex = spool.tile([P, 2, N], F16, tag="exp", bufs=8,
                                        name="ex")
                        if cp == 0:
                            # one wide exp across both banks (the unwritten
                            # [1, 0:128] corner is never read downstream)
                            nc.scalar.activation(ex[:], sc[:], AF.Exp,
                                                 scale=INV_SQRT_DK)
                        else:
                            nc.scalar.activation(ex[:, 0, 2 * P:],
                                                 sc[:, 0, 2 * P:], AF.Exp,
                                                 scale=INV_SQRT_DK)
                            nc.scalar.activation(ex[:, 1, 3 * P:],
                                                 sc[:, 1, 3 * P:], AF.Exp,
                                                 scale=INV_SQRT_DK)
                        for c2 in range(2):
                            cs = (2 * cp + c2) * P
                            nc.gpsimd.tensor_tensor(
                                ex[:, c2, cs:cs + P], ex[:, c2, cs:cs + P],
                                c01[:], op=ALU.mult)
                            exps.append(ex[:, c2, :])
                    if pend is not None:
                        sdp_ctx_fin(pend[0], pend[2])
                    pend = [h, exps, None]
                pend[2] = ctx_tile()
                for c in range(MCH):
                    sdp_ctx_mm(pend[0], pend[1], True, pend[2], c)
                sdp_ctx_fin(pend[0], pend[2])
                for cb in fillers.get(H, ()):
                    cb()

                def ev_self_o(j, ps):
                    nc.vector.tensor_tensor(xT[:, j], ps[:], xT[:, j],
                                            op=ALU.add)

                def lag1(hook):
                    # call hook(j-1) at step j: by then the j-1 eviction has
                    # drained, so the hoisted PE work doesn't stall the queue
                    return lambda j: hook(j - 1) if j > 0 else None

                # LN2 stats ride the self-O eviction stream
                ln2_state = ln_begin()
                proj(ctxT, wgrp(0, 3), ev_self_o, post=lag1(ln2_state[2]))
                ln2_state[2](DCH - 1)

                # ---- cross-attention (pipelined with Q projection; the
                # previous head pair's ctx matmuls interleave with the new
                # pair's score matmuls) ----
                ln_mid(hT, ln2_state[0])
                r2 = ln_rchain(ln2_state[0], ln2_state[1])
                wqx = wgrp(1, 0)

                def cross_pair_step(cpend, new_heads):
                    for ent in cpend:
                        ent[2] = ctx_tile()
                    new = [[h1, [], None, None] for h1 in new_heads]
                    for c in range(MCH):
                        for ent in cpend:
                            sdp_ctx_mm(ent[0], ent[1], False, ent[2], c)
                        for ent in new:
                            h1 = ent[0]
                            hc, off = h1 // 2, 64 * (h1 % 2)
                            if c % 2 == 0:
                                ent[3] = psum.tile([P, 2, N], F32, tag="sc",
                                                   bufs=2, name="sc")
                            sc = ent[3]
                            nc.tensor.matmul(
                                sc[:, c % 2],
                                kT[off:off + 64, hc, c * P:(c + 1) * P],
                                qT[off:off + 64, hc, :], start=True, stop=True)
                            if c % 2 == 1:
                                ex = spool.tile([P, 2, N], F16, tag="exp",
                                                bufs=8, name="ex")
                                nc.scalar.activation(ex[:], sc[:], AF.Exp,
                                                     scale=INV_SQRT_DK)
                                ent[1] += [ex[:, 0, :], ex[:, 1, :]]
                    for ent in cpend:
                        sdp_ctx_fin(ent[0], ent[2])
                    return new

                cpend = []
                for g in range(2):
                    wt = wpool.tile([P, DCH, N], F16, tag="wt", name="wt")
                    nc.sync.dma_start(wt[:, :, 0:N // 2], wqx[g][:, :, 0:N // 2])
                    nc.sync.dma_start(wt[:, :, N // 2:], wqx[g][:, :, N // 2:])
                    for jj in range(4):
                        j = g * 4 + jj
                        ps = psum.tile([P, N], F32, tag="mm", bufs=2, name="ps_cq")
                        for k in range(DCH):
                            nc.tensor.matmul(ps[:], wt[:, k, jj * P:(jj + 1) * P],
                                             hT[:, k], start=(k == 0),
                                             stop=(k == DCH - 1))
                        nc.vector.tensor_tensor(qT[:, j], ps[:], r2[:],
                                                op=ALU.mult)
                        cpend = cross_pair_step(cpend, (2 * j, 2 * j + 1))
                cross_pair_step(cpend, ())

                def ev_cross_o(j, ps):
                    nc.scalar.activation(cT[:, j], ps[:], AF.Copy)
                    nc.vector.tensor_tensor(xT[:, j], ps[:], xT[:, j],
                                            op=ALU.add)

                # lag 2: the DVE queue still holds the SDP tail here, so give
                # the hoisted stats matmuls two extra groups of slack
                ln3_state = ln_begin()
                proj(ctxT, wgrp(1, 3), ev_cross_o,
                     post=lambda j: ln3_state[2](j - 2) if j > 1 else None)
                ln3_state[2](DCH - 2)
                ln3_state[2](DCH - 1)

                # ---- FFN ----
                # ffn1 runs on x-mean; relu(r*z) = r*relu(z) since r > 0, so
                # the per-token r3 scale folds into the FFN2 eviction
                ln_mid(hT, ln3_state[0])
                r3 = ln_rchain(ln3_state[0], ln3_state[1])
                # FFN1: dense stream over 32 f-chunks
                for k in range(FCH):
                    e8, jj = k // 4, k % 4
                    if jj == 0:
                        wt = wpool.tile([P, DCH, N], F16, tag="wt", name="wt")
                        nc.sync.dma_start(wt[:, :, 0:N // 2],
                                          w1p[l, e8][:, :, 0:N // 2])
                        nc.sync.dma_start(wt[:, :, N // 2:],
                                          w1p[l, e8][:, :, N // 2:])
                        ck_state["w1"] = wt
                    wt = ck_state["w1"]
                    ps = psum.tile([P, N], F32, tag="mm", bufs=2, name="ps_f1")
                    for k8 in range(DCH):
                        nc.tensor.matmul(ps[:], wt[:, k8, jj * P:(jj + 1) * P],
                                         hT[:, k8], start=(k8 == 0),
                                         stop=(k8 == DCH - 1))
                    nc.scalar.activation(uT[:, k], ps[:], AF.Relu)

                # FFN2: j-major, one PSUM bank accumulates all 32 f-chunks.
                # post-eviction hooks: next layer's LN1 stats, or (last layer)
                # the final transposes for that d-chunk.
                if l < L - 1:
                    ln1_state = ln_begin()

                def ffn2_post(j):
                    if l < L - 1:
                        ln1_state[2](j)
                    else:
                        for m in range(MCH):
                            pst = psum.tile([P, 2 * N], F16, tag="mm", bufs=2,
                                            name="pst")
                            nc.tensor.transpose(pst[:, 0:P],
                                                xT[:, j, m * P:(m + 1) * P],
                                                ident[:])
                            tsb = spool.tile([P, P], F32, tag="osb", bufs=2,
                                             name="tsb")
                            nc.scalar.activation(tsb[:], pst[:, 0:P], AF.Copy)
                            nc.sync.dma_start(
                                out[m * P:(m + 1) * P, j * P:(j + 1) * P],
                                tsb[:])

                for j in range(DCH):
                    wt2 = wpool.tile([P, 8, 4, P], F16, tag="wt", name="wt2")
                    nc.sync.dma_start(wt2[:, 0:4], w2p[l, j][:, 0:4])
                    nc.sync.dma_start(wt2[:, 4:8], w2p[l, j][:, 4:8])
                    # last layer: transposes ride the mm tag, so keep jb on
                    # ctx (free there: no hoisted next-layer LN1 stats)
                    jb = psum.tile([P, N], F32, tag="ctx" if l == L - 1 else "mm",
                                   bufs=2, name="jb")
                    for k in range(FCH):
                        kb, ki = k // 4, k % 4
                        nc.tensor.matmul(jb[:], wt2[:, kb, ki], uT[:, k],
                                         start=(k == 0), stop=(k == FCH - 1),
                                         skip_group_check=True)
                    # x = cross_out + r3*ffn_hat (reference residual quirk,
                    # with the hoisted LN3 r-scale applied here)
                    nc.vector.tensor_tensor(xT[:, j], jb[:], r3[:],
                                            op=ALU.mult)
                    nc.vector.tensor_tensor(xT[:, j], xT[:, j], cT[:, j],
                                            op=ALU.add)
                    if j > 0:
                        ffn2_post(j - 1)
                ffn2_post(DCH - 1)

            work.release()

    nc.compile()
    return nc


# ----------------------------------------------------------------------------
# host-side packing
# ----------------------------------------------------------------------------

def _pack_T(aT):
    """[1024, C] (feature-major) -> tile image [128, 8, C] (fp16)."""
    d, c = aT.shape
    return np.ascontiguousarray(
        aT.reshape(DCH, P, c).transpose(1, 0, 2)).astype(np.float16)


def _pack_proj(w):
    """w [Dout, Din] (as in y = x @ w.T) -> [2, 128, 8, 512] group tile images."""
    wT = w.T  # [Din, Dout]
    return np.stack([_pack_T(wT[:, g * N:(g + 1) * N]) for g in range(2)])


def _prep(inputs):
    dec_inputs = np.asarray(inputs["dec_inputs"])
    self_mask = np.asarray(inputs["self_mask"])
    enc_output = np.asarray(inputs["enc_output"], dtype=np.float32)
    encoder_mask = np.asarray(inputs["encoder_mask"])
    embed = np.asarray(inputs["embed"], dtype=np.float32)
    pe = np.asarray(inputs["pe"], dtype=np.float32)
    wq, wk, wv, wo = (np.asarray(inputs[k], np.float32) for k in ("wq", "wk", "wv", "wo"))
    w1, w2 = np.asarray(inputs["ffn_w1"], np.float32), np.asarray(inputs["ffn_w2"], np.float32)

    # structural assumptions baked into the kernel
    causal_ref = np.triu(np.ones((LD, LD), bool), k=1)
    assert all(np.array_equal(self_mask[b], causal_ref) for b in range(B)), \
        "kernel assumes causal self mask"
    assert not encoder_mask.any(), "kernel assumes no encoder mask"
    for k in ("bq", "bk", "bv", "bo", "ffn_b1", "ffn_b2", "ln_b"):
        assert not np.asarray(inputs[k]).any(), f"kernel assumes zero {k}"
    assert np.all(np.asarray(inputs["ln_g"]) == 1.0), "kernel assumes unit ln gains"

    # shared (weight) arrays
    wp = np.empty((L, 2, 4, 2, P, DCH, N), np.float16)
    for l in range(L):
        for a in range(2):
            for i, w in enumerate((wq, wk, wv, wo)):
                wp[l, a, i] = _pack_proj(w[l, a])
    w1p = np.empty((L, 8, P, DCH, N), np.float16)
    w2p = np.empty((L, 8, P, 8, 4, P), np.float16)
    for l in range(L):
        w1T = w1[l].T  # [1024, 4096]
        for g in range(8):
            w1p[l, g] = _pack_T(w1T[:, g * N:(g + 1) * N])
        w2T = w2[l].T.astype(np.float16)  # [4096, 1024]
        # [j, p(f), kb, ki, dcol]: lhsT for (j, k=kb*4+ki) is
        # W2T[k*128:(k+1)*128, j*128:(j+1)*128]
        blk = w2T.reshape(8, 4, P, 8, P)   # [kb, ki, p, j, dcol]
        w2p[l] = np.ascontiguousarray(blk.transpose(3, 2, 0, 1, 4))

    causal01 = (~causal_ref[:P, :P]).astype(np.float16).T.copy()  # keep[tk, tq]

    shared = dict(wp=wp, w1p=w1p, w2p=w2p, causal=causal01)
    in_maps = []
    for b in range(B):
        x0 = embed[dec_inputs[b]] * np.float32(SQRT_D) + pe  # [512, 1024]
        m = dict(shared)
        m["x0p"] = _pack_T(np.ascontiguousarray(x0.T))
        m["encp"] = _pack_T(np.ascontiguousarray(enc_output[b].T))
        in_maps.append(m)
    return in_maps


def kernel(**inputs):
    if "nc" not in _CACHE:
        _CACHE["nc"] = _build_nc()
    nc = _CACHE["nc"]
    in_maps = _prep(inputs)

    from concourse import bass_utils
    trace = bool(int(os.environ.get("DECODER_TRACE", "0")))
    res = bass_utils.run_bass_kernel_spmd(
        nc, in_maps, core_ids=list(range(B)), trace=trace)
    _CACHE["last_result"] = res
    return np.stack([res.results[b]["out"] for b in range(B)]).astype(np.float32)


# revision 34
# speedup vs baseline: 1.0076x; 1.0076x over previous
"""Trainium2 Bass kernel for nn_Decoder_68289980006849 (3-layer transformer decoder).

Strategy: data-parallel over batch (B=8) across 8 NeuronCores; zero collectives.
Per core, the full decoder runs in "T-layout" [feature(partitions), token(free)]
with an fp16 datapath (fp32 PSUM accumulation):

  - x = embed-gather*sqrt(D) + pe precomputed on host (input sharding)
  - LayerNorm split three ways: per-chunk stats matmuls are hoisted into the
    previous phase's residual evictions; projections run on (x - mean) as soon
    as the mean lands; the 1/std factor is computed in parallel and folded
    into each projection's PSUM eviction (for the FFN it rides the FFN2
    eviction, exact since relu(r*z) = r*relu(z) for r > 0)
  - attention: scores computed transposed (scores_T[tk, tq]) so the softmaxed
    matrix feeds the context matmul directly; heads are software-pipelined
    (head h-1's context matmuls interleave MM-by-MM with head h's score
    matmuls); score chunk-pairs land in one 2-bank fp32 PSUM tile and get a
    single wide ACT exp; 64 ones-columns prepended to V give the softmax
    denominator rows at PSUM partition 0, where the fast reciprocal reads
    PSUM directly; one fused TT normalizes each head's context; causal
    masking via column-slicing + diagonal-block multiply on GpSimd
  - cross-attn K/V projections run as fillers inside the self-attn SDP phase
  - FFN: dense FFN1 stream, then FFN2 j-major with full PSUM accumulation
    (one fused eviction per d-chunk incl. the reference quirk x = cross + ffn);
    cross-attn output stays resident in SBUF
  - final transposes interleave with the last layer's FFN2 evictions
  - weights pre-packed on host as fp16 tile images (contiguous 0.5-1MB DMAs);
    fp16 halves HBM traffic and enables fast weight load on the PE

Self-contained: only stdlib + numpy + the concourse/bass stack on PYTHONPATH.
"""

import os
import numpy as np

import concourse.bass as bass
import concourse.tile as tile
from concourse import bacc, mybir
from concourse.masks import make_identity

# ---- problem constants (hardcoded per contract) ----
B, LD, LE = 8, 512, 512
D, H, DK, F, L, V = 1024, 16, 64, 4096, 3, 32000
M = LD                      # tokens per core
DCH = D // 128              # 8 d-model chunks
FCH = F // 128              # 32 ffn chunks
MCH = M // 128              # 4 token chunks
SQRT_D = 32.0
INV_SQRT_DK = 0.125
EPS = 1e-5
NONES = 64                  # ones-columns appended to V (denominator rows)

P = 128
N = 512
F32 = mybir.dt.float32
F16 = mybir.dt.float16
AF = mybir.ActivationFunctionType
ALU = mybir.AluOpType

_CACHE = {}


# ----------------------------------------------------------------------------
# Bass program (identical on all 8 cores; data differs via in_maps)
# ----------------------------------------------------------------------------

def _build_nc():
    nc = bacc.Bacc("TRN2", target_bir_lowering=False, debug=False,
                   enable_asserts=False, num_devices=8)

    # inputs (per core); x0p = embed-gather * sqrt(D) + pe, done on host
    x0p = nc.dram_tensor("x0p", [P, DCH, N], F16, kind="ExternalInput").ap()
    encp = nc.dram_tensor("encp", [P, DCH, N], F16, kind="ExternalInput").ap()
    # projection weights, packed: [l, a, i(q,k,v,o), g, 128, 8, 512]
    wp = nc.dram_tensor("wp", [L, 2, 4, 2, P, DCH, N], F16, kind="ExternalInput").ap()
    w1p = nc.dram_tensor("w1p", [L, 8, P, DCH, N], F16, kind="ExternalInput").ap()
    # FFN2 weights j-major: [l, j, 128(f), 8(kb), 4(ki), 128(d)]
    w2p = nc.dram_tensor("w2p", [L, 8, P, 8, 4, P], F16, kind="ExternalInput").ap()
    causal = nc.dram_tensor("causal", [P, P], F16, kind="ExternalInput").ap()
    out = nc.dram_tensor("out", [M, D], F32, kind="ExternalOutput").ap()

    with tile.TileContext(nc) as tc:
        with tc.tile_pool(name="res", bufs=1) as res, \
             tc.tile_pool(name="wpool", bufs=4) as wpool, \
             tc.tile_pool(name="spool", bufs=2) as spool, \
             tc.tile_pool(name="psum", bufs=1, space="PSUM") as psum:

            # ---- resident tiles ----
            xT = res.tile([P, DCH, N], F16)         # residual stream
            encT = res.tile([P, DCH, N], F16)       # encoder output (transposed)
            Vst = res.tile([P, MCH, H, 64 + NONES], F16)  # V + ones columns
            Jsc = res.tile([P, P], F16)             # all-(1/D) for LN stats
            c01 = res.tile([P, P], F16)             # causal diagonal 0/1 keep-mask
            ident = res.tile([P, P], F16)
            epsc = res.tile([P, 1], F32)

            nc.vector.memset(epsc[:], EPS)
            nc.vector.memset(Jsc[:], 1.0 / D)
            # ones-columns FIRST: the denominator rows then land at PSUM
            # partition base 0, where the direct-from-PSUM reciprocal works
            nc.vector.memset(Vst[:, :, :, 0:NONES], 1.0)

            # ---- x = embed*sqrt(D) + pe, precomputed on host; per-chunk DMA
            # so the first LayerNorm stats can start early ----
            for c in range(DCH):
                nc.sync.dma_start(xT[:, c], x0p[:, c])

            # needed from the first SDP phase onwards, not at start
            nc.sync.dma_start(encT[:], encp)
            nc.sync.dma_start(c01[:], causal)
            make_identity(nc, ident[:])

            work = tc.alloc_tile_pool(name="work", bufs=1)
            hT = work.tile([P, DCH, N], F16, tag="hT")
            hS = work.tile([P, DCH, N], F16, tag="hS")
            ctxT = work.tile([P, DCH, N], F16, tag="ctxT")
            qT = work.tile([P, DCH, N], F16, tag="qT")
            kT = work.tile([P, DCH, N], F16, tag="kT")
            cT = work.tile([P, DCH, N], F16, tag="cT")    # cross-attn output
            uT = work.tile([P, FCH, N], F16, tag="uT")    # relu(ffn1) activations

            # ---- LayerNorm split: stats (hoistable) + finish (chain+apply) ----
            def ln_begin():
                mean_ps = psum.tile([P, N], F32, tag="ctx", bufs=2, name="mean_ps")
                msq_ps = psum.tile([P, N], F32, tag="ctx", bufs=2, name="msq_ps")

                def stat_chunk(c):
                    sq = spool.tile([P, N], F16, tag="sq", bufs=2, name="sq")
                    nc.scalar.activation(sq[:], xT[:, c], AF.Square)
                    nc.tensor.matmul(mean_ps[:], Jsc[:], xT[:, c],
                                     start=(c == 0), stop=(c == DCH - 1),
                                     skip_group_check=True)
                    nc.tensor.matmul(msq_ps[:], Jsc[:], sq[:],
                                     start=(c == 0), stop=(c == DCH - 1),
                                     skip_group_check=True)
                return mean_ps, msq_ps, stat_chunk

            def ln_mid(dst, mean_ps):
                """dst = x - mean: the minimal chain before projections can
                start (r-scaling is folded into their evictions)."""
                mSB = spool.tile([P, N], F16, tag="stt", name="mSB")
                nc.scalar.activation(mSB[:], mean_ps[:], AF.Copy)
                for c in range(DCH):
                    nc.vector.tensor_tensor(dst[:, c], xT[:, c], mSB[:],
                                            op=ALU.subtract)

            def ln_rchain(mean_ps, msq_ps):
                """rstd (fp16, broadcast) - overlaps the next phase's matmuls."""
                m2 = spool.tile([P, N], F32, tag="stt32", name="m2")
                nc.scalar.activation(m2[:], mean_ps[:], AF.Square)
                var = spool.tile([P, N], F32, tag="stt32", name="var")
                nc.vector.tensor_tensor(var[:], msq_ps[:], m2[:], op=ALU.subtract)
                sd = spool.tile([P, N], F32, tag="stt32", name="sd")
                nc.scalar.activation(sd[:], var[:], AF.Sqrt, bias=epsc[:])
                rstd = spool.tile([P, N], F32, tag="stt32", name="rstd")
                nc.vector.reciprocal_approx_fast(rstd[:], sd[:])
                rstdh = spool.tile([P, N], F16, tag="stt", name="rstdh")
                nc.vector.tensor_copy(rstdh[:], rstd[:])
                return rstdh

            def proj(rhs, w_groups, evict, post=None):
                """out[j] = sum_k W[k, j-chunk].T @ rhs[k]; W streamed in 1MB
                tiles, landing in column halves for early start."""
                for g in range(2):
                    wt = wpool.tile([P, DCH, N], F16, tag="wt", name="wt")
                    nc.sync.dma_start(wt[:, :, 0:N // 2], w_groups[g][:, :, 0:N // 2])
                    nc.sync.dma_start(wt[:, :, N // 2:], w_groups[g][:, :, N // 2:])
                    for jj in range(4):
                        j = g * 4 + jj
                        ps = psum.tile([P, N], F32, tag="mm", bufs=2, name="ps_mm")
                        for k in range(DCH):
                            nc.tensor.matmul(ps[:], wt[:, k, jj * P:(jj + 1) * P],
                                             rhs[:, k], start=(k == 0),
                                             stop=(k == DCH - 1))
                        evict(j, ps)
                        if post is not None:
                            post(j)

            def v_proj(kv_rhs, w_groups):
                """V_nat[tk, dv] -> Vst[:, t, h, :64] slices."""
                for g in range(2):  # dv halves (heads g*8..g*8+7)
                    wt = wpool.tile([P, DCH, N], F16, tag="wt", name="wt")
                    nc.sync.dma_start(wt[:], w_groups[g])
                    for t in range(MCH):
                        ps = psum.tile([P, N], F32, tag="mm", bufs=2, name="ps_v")
                        for k in range(DCH):
                            nc.tensor.matmul(ps[:], kv_rhs[:, k, t * P:(t + 1) * P],
                                             wt[:, k], start=(k == 0),
                                             stop=(k == DCH - 1))
                        nc.scalar.activation(
                            Vst[:, t, g * 8:(g + 1) * 8, NONES:NONES + 64],
                            ps[:].rearrange("p (h d) -> p h d", d=64), AF.Copy)

            # ---- attention: scores/ctx split for head software-pipelining ----
            def sdp_ctx_mm(h, exps, is_self, ctx, c):
                nc.tensor.matmul(ctx[:, (c * P if is_self else 0):],
                                 Vst[:, c, h, :],
                                 exps[c][:, (c * P if is_self else 0):],
                                 start=(c == 0), stop=(c == MCH - 1),
                                 skip_group_check=True)

            def sdp_ctx_fin(h, ctx):
                hc, off = h // 2, 64 * (h % 2)
                rec = spool.tile([NONES, N], F32, tag="rec", bufs=2, name="rec")
                nc.vector.reciprocal_approx_fast(rec[:], ctx[0:NONES, :])
                nc.vector.tensor_tensor(ctxT[off:off + 64, hc, :],
                                        ctx[NONES:NONES + 64, :],
                                        rec[:], op=ALU.mult)

            def ctx_tile():
                return psum.tile([P, N], F32, tag="ctx", bufs=2, name="ctx")

            for l in range(L):
                def wgrp(a, i):
                    return [wp[l, a, i, g] for g in range(2)]

                # ---- self-attention ----
                if l == 0:
                    ln1_state = ln_begin()
                    for c in range(DCH):
                        ln1_state[2](c)
                ln_mid(hT, ln1_state[0])
                r1 = ln_rchain(ln1_state[0], ln1_state[1])

                def ev_scale(dst, r):
                    return lambda j, ps: nc.vector.tensor_tensor(
                        dst[:, j], ps[:], r[:], op=ALU.mult)
                proj(hT, wgrp(0, 0), ev_scale(qT, r1))
                # hS = full normalized h (for the V projection, which needs
                # per-token scaling on its matmul INPUT side)
                for c in range(DCH):
                    nc.vector.tensor_tensor(hS[:, c], hT[:, c], r1[:],
                                            op=ALU.mult)
                proj(hT, wgrp(0, 1), ev_scale(kT, r1))
                v_proj(hS, wgrp(0, 2))

                # cross-attn K/V projections depend only on encT + weights ->
                # emitted as fillers inside the self-attention SDP.
                ck_state = {}
                wkx, wvx = wgrp(1, 1), wgrp(1, 2)

                def ck_dma(g):
                    def f():
                        wt = wpool.tile([P, DCH, N], F16, tag="wt", name="wt")
                        nc.sync.dma_start(wt[:, :, 0:N // 2], wkx[g][:, :, 0:N // 2])
                        nc.sync.dma_start(wt[:, :, N // 2:], wkx[g][:, :, N // 2:])
                        ck_state["k"] = wt
                    return f

                def ck_chunk(g, jj):
                    def f():
                        wt = ck_state["k"]
                        j = g * 4 + jj
                        ps = psum.tile([P, N], F32, tag="mm", bufs=2, name="ps_ck")
                        for k in range(DCH):
                            nc.tensor.matmul(ps[:], wt[:, k, jj * P:(jj + 1) * P],
                                             encT[:, k], start=(k == 0),
                                             stop=(k == DCH - 1))
                        nc.vector.tensor_copy(kT[:, j], ps[:])
                    return f

                def cv_dma(g):
                    def f():
                        wt = wpool.tile([P, DCH, N], F16, tag="wt", name="wt")
                        nc.sync.dma_start(wt[:], wvx[g])
                        ck_state["v"] = wt
                    return f

                def cv_chunk(g, t):
                    def f():
                        wt = ck_state["v"]
                        ps = psum.tile([P, N], F32, tag="mm", bufs=2, name="ps_cv")
                        for k in range(DCH):
                            nc.tensor.matmul(ps[:], encT[:, k, t * P:(t + 1) * P],
                                             wt[:, k], start=(k == 0),
                                             stop=(k == DCH - 1))
                        nc.vector.tensor_copy(
                            Vst[:, t, g * 8:(g + 1) * 8, NONES:NONES + 64],
                            ps[:].rearrange("p (h d) -> p h d", d=64))
                    return f

                # kT[:, j] may be overwritten only after heads 2j, 2j+1 read
                # it; Vst heads g*8..g*8+7 only after those heads' ctx matmuls.
                fillers = {
                    0: [ck_dma(0)],
                    2: [ck_chunk(0, 0)], 4: [ck_chunk(0, 1)],
                    6: [ck_chunk(0, 2)], 8: [ck_chunk(0, 3), ck_dma(1)],
                    10: [ck_chunk(1, 0), cv_dma(0)],
                    12: [ck_chunk(1, 1), cv_chunk(0, 0)],
                    13: [cv_chunk(0, 1)],
                    14: [ck_chunk(1, 2), cv_chunk(0, 2)],
                    15: [cv_chunk(0, 3)],
                    16: [ck_chunk(1, 3), cv_dma(1),
                         cv_chunk(1, 0), cv_chunk(1, 1),
                         cv_chunk(1, 2), cv_chunk(1, 3)],
                }

                # head-pipelined self SDP: head h-1's ctx matmuls interleave
                # MM-by-MM with head h's score matmuls (hides the short-score
                # LDWEIGHTS exposure and the exp chain)
                pend = None
                for h in range(H):
                    for cb in fillers.get(h, ()):
                        cb()
                    hc, off = h // 2, 64 * (h % 2)
                    exps = []
                    if pend is not None:
                        pend[2] = ctx_tile()
                    for cp in range(2):  # chunk pairs in one 2-bank PSUM tile
                        sc = psum.tile([P, 2, N], F32, tag="sc", bufs=2,
                                       name="sc")
                        for c2 in range(2):
                            c = 2 * cp + c2
                            cs = c * P
                            nc.tensor.matmul(sc[:, c2, cs:],
                                             kT[off:off + 64, hc,
                                                c * P:(c + 1) * P],
                                             qT[off:off + 64, hc, cs:],
                                             start=True, stop=True)
                            if pend is not None:
                                sdp_ctx_mm(pend[0], pend[1], True, pend[2], c)
                        ex = spool.tile([P, 2, N], F16, tag="exp", bufs=8,
                                        name="ex")
                        if cp == 0:
                            # one wide exp across both banks (the unwritten
                            # [1, 0:128] corner is never read downstream)
                            nc.scalar.activation(ex[:], sc[:], AF.Exp,
                                                 scale=INV_SQRT_DK)
                        else:
                            nc.scalar.activation(ex[:, 0, 2 * P:],
                                                 sc[:, 0, 2 * P:], AF.Exp,
                                                 scale=INV_SQRT_DK)
                            nc.scalar.activation(ex[:, 1, 3 * P:],
                                                 sc[:, 1, 3 * P:], AF.Exp,
                                                 scale=INV_SQRT_DK)
                        for c2 in range(2):
                            cs = (2 * cp + c2) * P
                            nc.gpsimd.tensor_tensor(
                                ex[:, c2, cs:cs + P], ex[:, c2, cs:cs + P],
                                c01[:], op=ALU.mult)
                            exps.append(ex[:, c2, :])
                    if pend is not None:
                        sdp_ctx_fin(pend[0], pend[2])
                    pend = [h, exps, None]
                pend[2] = ctx_tile()
                for c in range(MCH):
                    sdp_ctx_mm(pend[0], pend[1], True, pend[2], c)
                sdp_ctx_fin(pend[0], pend[2])
                for cb in fillers.get(H, ()):
                    cb()

                def ev_self_o(j, ps):
                    nc.vector.tensor_tensor(xT[:, j], ps[:], xT[:, j],
                                            op=ALU.add)

                def lag1(hook):
                    # call hook(j-1) at step j: by then the j-1 eviction has
                    # drained, so the hoisted PE work doesn't stall the queue
                    return lambda j: hook(j - 1) if j > 0 else None

                # LN2 stats ride the self-O eviction stream
                ln2_state = ln_begin()
                proj(ctxT, wgrp(0, 3), ev_self_o, post=lag1(ln2_state[2]))
                ln2_state[2](DCH - 1)

                # ---- cross-attention (pipelined with Q projection; the
                # previous head pair's ctx matmuls interleave with the new
                # pair's score matmuls) ----
                ln_mid(hT, ln2_state[0])
                r2 = ln_rchain(ln2_state[0], ln2_state[1])
                wqx = wgrp(1, 0)

                def cross_pair_step(cpend, new_heads):
                    for ent in cpend:
                        ent[2] = ctx_tile()
                    new = [[h1, [], None, None] for h1 in new_heads]
                    for c in range(MCH):
                        for ent in cpend:
                            sdp_ctx_mm(ent[0], ent[1], False, ent[2], c)
                        for ent in new:
                            h1 = ent[0]
                            hc, off = h1 // 2, 64 * (h1 % 2)
                            if c % 2 == 0:
                                ent[3] = psum.tile([P, 2, N], F32, tag="sc",
                                                   bufs=2, name="sc")
                            sc = ent[3]
                            nc.tensor.matmul(
                                sc[:, c % 2],
                                kT[off:off + 64, hc, c * P:(c + 1) * P],
                                qT[off:off + 64, hc, :], start=True, stop=True)
                            if c % 2 == 1:
                                ex = spool.tile([P, 2, N], F16, tag="exp",
                                                bufs=8, name="ex")
                                nc.scalar.activation(ex[:], sc[:], AF.Exp,
                                                     scale=INV_SQRT_DK)
                                ent[1] += [ex[:, 0, :], ex[:, 1, :]]
                    for ent in cpend:
                        sdp_ctx_fin(ent[0], ent[2])
                    return new

                cpend = []
                for g in range(2):
                    wt = wpool.tile([P, DCH, N], F16, tag="wt", name="wt")
                    nc.sync.dma_start(wt[:, :, 0:N // 2], wqx[g][:, :, 0:N // 2])
                    nc.sync.dma_start(wt[:, :, N // 2:], wqx[g][:, :, N // 2:])
                    for jj in range(4):
                        j = g * 4 + jj
                        ps = psum.tile([P, N], F32, tag="mm", bufs=2, name="ps_cq")
                        for k in range(DCH):
                            nc.tensor.matmul(ps[:], wt[:, k, jj * P:(jj + 1) * P],
                                             hT[:, k], start=(k == 0),
                                             stop=(k == DCH - 1))
                        nc.vector.tensor_tensor(qT[:, j], ps[:], r2[:],
                                                op=ALU.mult)
                        cpend = cross_pair_step(cpend, (2 * j, 2 * j + 1))
                cross_pair_step(cpend, ())

                def ev_cross_o(j, ps):
                    nc.scalar.activation(cT[:, j], ps[:], AF.Copy)
                    nc.vector.tensor_tensor(xT[:, j], ps[:], xT[:, j],
                                            op=ALU.add)

                # lag 2: the DVE queue still holds the SDP tail here, so give
                # the hoisted stats matmuls two extra groups of slack
                ln3_state = ln_begin()
                proj(ctxT, wgrp(1, 3), ev_cross_o,
                     post=lambda j: ln3_state[2](j - 2) if j > 1 else None)
                ln3_state[2](DCH - 2)
                ln3_state[2](DCH - 1)

                # ---- FFN ----
                # ffn1 runs on x-mean; relu(r*z) = r*relu(z) since r > 0, so
                # the per-token r3 scale folds into the FFN2 eviction
                ln_mid(hT, ln3_state[0])
                r3 = ln_rchain(ln3_state[0], ln3_state[1])
                # FFN1: dense stream over 32 f-chunks
                for k in range(FCH):
                    e8, jj = k // 4, k % 4
                    if jj == 0:
                        wt = wpool.tile([P, DCH, N], F16, tag="wt", name="wt")
                        nc.sync.dma_start(wt[:, :, 0:N // 2],
                                          w1p[l, e8][:, :, 0:N // 2])
                        nc.sync.dma_start(wt[:, :, N // 2:],
                                          w1p[l, e8][:, :, N // 2:])
                        ck_state["w1"] = wt
                    wt = ck_state["w1"]
                    ps = psum.tile([P, N], F32, tag="mm", bufs=2, name="ps_f1")
                    for k8 in range(DCH):
                        nc.tensor.matmul(ps[:], wt[:, k8, jj * P:(jj + 1) * P],
                                         hT[:, k8], start=(k8 == 0),
                                         stop=(k8 == DCH - 1))
                    nc.scalar.activation(uT[:, k], ps[:], AF.Relu)

                # FFN2: j-major, one PSUM bank accumulates all 32 f-chunks.
                # post-eviction hooks: next layer's LN1 stats, or (last layer)
                # the final transposes for that d-chunk.
                if l < L - 1:
                    ln1_state = ln_begin()

                def ffn2_post(j):
                    if l < L - 1:
                        ln1_state[2](j)
                    else:
                        for m in range(MCH):
                            pst = psum.tile([P, 2 * N], F16, tag="mm", bufs=2,
                                            name="pst")
                            nc.tensor.transpose(pst[:, 0:P],
                                                xT[:, j, m * P:(m + 1) * P],
                                                ident[:])
                            tsb = spool.tile([P, P], F32, tag="osb", bufs=2,
                                             name="tsb")
                            nc.scalar.activation(tsb[:], pst[:, 0:P], AF.Copy)
                            nc.sync.dma_start(
                                out[m * P:(m + 1) * P, j * P:(j + 1) * P],
                                tsb[:])

                for j in range(DCH):
                    wt2 = wpool.tile([P, 8, 4, P], F16, tag="wt", name="wt2")
                    nc.sync.dma_start(wt2[:, 0:4], w2p[l, j][:, 0:4])
                    nc.sync.dma_start(wt2[:, 4:8], w2p[l, j][:, 4:8])
                    # last layer: transposes ride the mm tag, so keep jb on
                    # ctx (free there: no hoisted next-layer LN1 stats)
                    jb = psum.tile([P, N], F32, tag="ctx" if l == L - 1 else "mm",
                                   bufs=2, name="jb")
                    for k in range(FCH):
                        kb, ki = k // 4, k % 4
                        nc.tensor.matmul(jb[:], wt2[:, kb, ki], uT[:, k],
                                         start=(k == 0), stop=(k == FCH - 1),
                                         skip_group_check=True)
                    # x = cross_out + r3*ffn_hat (reference residual quirk,
                    # with the hoisted LN3 r-scale applied here)
                    nc.vector.tensor_tensor(xT[:, j], jb[:], r3[:],
                                            op=ALU.mult)
                    nc.vector.tensor_tensor(xT[:, j], xT[:, j], cT[:, j],
                                            op=ALU.add)
                    if j > 0:
                        ffn2_post(j - 1)
                ffn2_post(DCH - 1)

            work.release()

    nc.compile()
    return nc


# ----------------------------------------------------------------------------
# host-side packing
# ----------------------------------------------------------------------------

def _pack_T(aT):
    """[1024, C] (feature-major) -> tile image [128, 8, C] (fp16)."""
    d, c = aT.shape
    return np.ascontiguousarray(
        aT.reshape(DCH, P, c).transpose(1, 0, 2)).astype(np.float16)


def _pack_proj(w):
    """w [Dout, Din] (as in y = x @ w.T) -> [2, 128, 8, 512] group tile images."""
    wT = w.T  # [Din, Dout]
    return np.stack([_pack_T(wT[:, g * N:(g + 1) * N]) for g in range(2)])


def _prep(inputs):
    dec_inputs = np.asarray(inputs["dec_inputs"])
    self_mask = np.asarray(inputs["self_mask"])
    enc_output = np.asarray(inputs["enc_output"], dtype=np.float32)
    encoder_mask = np.asarray(inputs["encoder_mask"])
    embed = np.asarray(inputs["embed"], dtype=np.float32)
    pe = np.asarray(inputs["pe"], dtype=np.float32)
    wq, wk, wv, wo = (np.asarray(inputs[k], np.float32) for k in ("wq", "wk", "wv", "wo"))
    w1, w2 = np.asarray(inputs["ffn_w1"], np.float32), np.asarray(inputs["ffn_w2"], np.float32)

    # structural assumptions baked into the kernel
    causal_ref = np.triu(np.ones((LD, LD), bool), k=1)
    assert all(np.array_equal(self_mask[b], causal_ref) for b in range(B)), \
        "kernel assumes causal self mask"
    assert not encoder_mask.any(), "kernel assumes no encoder mask"
    for k in ("bq", "bk", "bv", "bo", "ffn_b1", "ffn_b2", "ln_b"):
        assert not np.asarray(inputs[k]).any(), f"kernel assumes zero {k}"
    assert np.all(np.asarray(inputs["ln_g"]) == 1.0), "kernel assumes unit ln gains"

    # shared (weight) arrays
    wp = np.empty((L, 2, 4, 2, P, DCH, N), np.float16)
    for l in range(L):
        for a in range(2):
            for i, w in enumerate((wq, wk, wv, wo)):
                wp[l, a, i] = _pack_proj(w[l, a])
    w1p = np.empty((L, 8, P, DCH, N), np.float16)
    w2p = np.empty((L, 8, P, 8, 4, P), np.float16)
    for l in range(L):
        w1T = w1[l].T  # [1024, 4096]
        for g in range(8):
            w1p[l, g] = _pack_T(w1T[:, g * N:(g + 1) * N])
        w2T = w2[l].T.astype(np.float16)  # [4096, 1024]
        # [j, p(f), kb, ki, dcol]: lhsT for (j, k=kb*4+ki) is
        # W2T[k*128:(k+1)*128, j*128:(j+1)*128]
        blk = w2T.reshape(8, 4, P, 8, P)   # [kb, ki, p, j, dcol]
        w2p[l] = np.ascontiguousarray(blk.transpose(3, 2, 0, 1, 4))

    causal01 = (~causal_ref[:P, :P]).astype(np.float16).T.copy()  # keep[tk, tq]

    shared = dict(wp=wp, w1p=w1p, w2p=w2p, causal=causal01)
    in_maps = []
    for b in range(B):
        x0 = embed[dec_inputs[b]] * np.float32(SQRT_D) + pe  # [512, 1024]
        m = dict(shared)
        m["x0p"] = _pack_T(np.ascontiguousarray(x0.T))
        m["encp"] = _pack_T(np.ascontiguousarray(enc_output[b].T))
        in_maps.append(m)
    return in_maps


def kernel(**inputs):
    if "nc" not in _CACHE:
        _CACHE["nc"] = _build_nc()
    nc = _CACHE["nc"]
    in_maps = _prep(inputs)

    from concourse import bass_utils
    trace = bool(int(os.environ.get("DECODER_TRACE", "0")))
    res = bass_utils.run_bass_kernel_spmd(
        nc, in_maps, core_ids=list(range(B)), trace=trace)
    _CACHE["last_result"] = res
    return np.stack([res.results[b]["out"] for b in range(B)]).astype(np.float32)
